# revision 14
# baseline (speedup 1.0000x reference)
"""Trainium2 Bass kernel: AdaptiveAttentionFusion, pure data-parallel on 8 NeuronCores.

Strategy:
  - Shard batch B=16384 across 8 cores (2048 rows each); weights replicated.
  - Host-side weight folding: input projections absorbed into QKV / MLP weights
    (q_f = frontier @ (Wf@Wq) + ...), so raw inputs feed all matmuls directly.
  - All matmul operands fp16 (validated 2.9e-4 rel err), fp32 PSUM accumulation.
  - Natural layout (rows on partitions); lhsT = transposed activations via PE
    transposes packed 4-per-PSUM-bank; biases folded in via K=1 ones-row matmuls.
  - Single ACT table set (natural_log_exp): softmax exp; LN rstd = exp(-0.5*ln(var+eps));
    sigmoid = exp(-ln(1+exp(-x))).
"""
import os
import numpy as np

import concourse.bacc as bacc
import concourse.bass as bass
import concourse.tile as tile
from concourse import mybir
from concourse.bass_utils import run_bass_kernel_spmd
from concourse.masks import make_identity

D, H, KD = 512, 4, 128
NCORES = 8
EPS = 1e-6
P = 128
F16 = mybir.dt.float16
F32 = mybir.dt.float32
AF = mybir.ActivationFunctionType
OP = mybir.AluOpType
AX = mybir.AxisListType

LAST_EXEC_TIME_NS = None
LAST_RESULTS = None


def ts(i, n=P):
    return slice(i * n, (i + 1) * n)


def _fold(inp):
    """Fold input projections into downstream weights. Returns (device arrays, ba2, affine)."""
    Ws = [inp['Wf'], inp['Wr'], inp['Wm']]
    bs = [inp['bf'], inp['br'], inp['bm']]
    Wqkv = np.concatenate([inp['Wq'].reshape(D, D), inp['Wk'].reshape(D, D),
                           inp['Wv'].reshape(D, D)], axis=1)          # [512, 1536]
    bqkv = np.concatenate([inp['bq'].reshape(-1), inp['bk'].reshape(-1),
                           inp['bv'].reshape(-1)])                    # [1536]
    f = {}
    f['wproj'] = np.concatenate(Ws, axis=1).astype(np.float16)        # [512, 1536]
    f['bproj'] = np.concatenate(bs)[None, :].astype(np.float16)       # [1, 1536]
    f['wqkv'] = np.concatenate([Ws[s] @ Wqkv for s in range(3)], axis=1).astype(np.float16)  # [512, 4608]
    f['bqkv'] = np.concatenate([bs[s] @ Wqkv + bqkv for s in range(3)])[None, :].astype(np.float16)
    A = [Ws[s] @ inp['Wa1'][s * D:(s + 1) * D] for s in range(3)]
    f['wh'] = np.concatenate(A, axis=1).astype(np.float16)            # [512, 768]
    f['bh'] = (inp['ba1'] + sum(bs[s] @ inp['Wa1'][s * D:(s + 1) * D]
                                for s in range(3)))[None, :].astype(np.float16)
    f['wo'] = inp['Wo'].reshape(D, D).astype(np.float16)
    f['bo'] = inp['bo'][None, :].astype(np.float16)
    f['wg'] = inp['Wg'].astype(np.float16)
    f['bg'] = inp['bg'][None, :].astype(np.float16)
    f['wa2b'] = np.ascontiguousarray(inp['Wa2'].T).astype(np.float32)  # [3, 256]
    aff = np.stack([inp['gamma1'], inp['beta1'], inp['gamma2'], inp['beta2']]).astype(np.float32)
    return f, inp['ba2'].astype(np.float32), aff


def _pin_act_table(nc):
    """Instance-level override of insert_act_table_loads: make
    natural_log_exp_and_others the only selectable ACT table set, so the
    kernel never thrashes table loads (it covers exp/ln/relu/copy/identity).
    Set ids stay aligned with act_info.json ordering."""
    import types
    import bass_rust as _bass_rust
    from concourse.hw_specs import get_activation_tables

    def patched(self):
        has_activation = any(
            isinstance(i, mybir.InstActivation)
            for b in self.main_func.blocks
            for i in b.instructions
        )
        if not has_activation:
            return
        tables = [
            (name, fns if name == "natural_log_exp_and_others" else set())
            for name, fns in get_activation_tables(self.m.arch).items()
        ]
        _bass_rust.insert_act_table_loads(self, tables)

    nc.insert_act_table_loads = types.MethodType(patched, nc)


def _build(R, ba2, need_aff1, need_aff2):
    ntiles = R // P
    nc = bacc.Bacc()
    _pin_act_table(nc)

    x_ext = [nc.declare_dram_parameter(n, [R, D], F32, isOutput=False)
             for n in ("frontier", "cross_robot", "map_feat")]
    wproj_d = nc.declare_dram_parameter("wproj", [D, 3 * D], F16, isOutput=False)
    bproj_d = nc.declare_dram_parameter("bproj", [1, 3 * D], F16, isOutput=False)
    wqkv_d = nc.declare_dram_parameter("wqkv", [D, 9 * D], F16, isOutput=False)
    bqkv_d = nc.declare_dram_parameter("bqkv", [1, 9 * D], F16, isOutput=False)
    wh_d = nc.declare_dram_parameter("wh", [D, 768], F16, isOutput=False)
    bh_d = nc.declare_dram_parameter("bh", [1, 256], F16, isOutput=False)
    wo_d = nc.declare_dram_parameter("wo", [D, D], F16, isOutput=False)
    bo_d = nc.declare_dram_parameter("bo", [1, D], F16, isOutput=False)
    wg_d = nc.declare_dram_parameter("wg", [D, D], F16, isOutput=False)
    bg_d = nc.declare_dram_parameter("bg", [1, D], F16, isOutput=False)
    wa2b_d = nc.declare_dram_parameter("wa2b", [3, 256], F32, isOutput=False)
    ba2_d = nc.declare_dram_parameter("ba2", [1, 3], F32, isOutput=False)
    aff_d = None
    if need_aff1 or need_aff2:
        aff_d = nc.declare_dram_parameter("aff", [4, D], F32, isOutput=False)
    out_ext = nc.declare_dram_parameter("out", [R, D], F32, isOutput=True)

    def bcast(ap, parts=P):
        """Partition-broadcast DMA source AP (stride-0 partition dim)."""
        return bass.AP(tensor=ap.tensor, offset=ap.offset, ap=[[0, parts]] + list(ap.ap))

    with tile.TileContext(nc) as tc:
        import contextlib
        with contextlib.ExitStack() as ctx:
            const = ctx.enter_context(tc.tile_pool(name="const", bufs=1))
            p_raw = ctx.enter_context(tc.tile_pool(name="p_raw", bufs=2))
            p_rt = ctx.enter_context(tc.tile_pool(name="p_rt", bufs=2))
            p_x = ctx.enter_context(tc.tile_pool(name="p_x", bufs=2))
            p_qkv = ctx.enter_context(tc.tile_pool(name="p_qkv", bufs=2))
            p_ctx = ctx.enter_context(tc.tile_pool(name="p_ctx", bufs=2))
            p_y = ctx.enter_context(tc.tile_pool(name="p_y", bufs=2))
            p_w = ctx.enter_context(tc.tile_pool(name="p_w", bufs=2))
            p_sm = ctx.enter_context(tc.tile_pool(name="p_sm", bufs=2))
            ps_t32 = ctx.enter_context(tc.tile_pool(name="ps_t32", bufs=2, space="PSUM"))
            ps_t16 = ctx.enter_context(tc.tile_pool(name="ps_t16", bufs=2, space="PSUM"))
            ps_mm = ctx.enter_context(tc.tile_pool(name="ps_mm", bufs=3, space="PSUM"))

            # ---- constants ----
            wproj_sb = const.tile([P, 4, 3 * D], F16)
            nc.sync.dma_start(out=wproj_sb, in_=wproj_d[:, :].rearrange("(c p) f -> p c f", p=P))
            bproj_sb = const.tile([1, 3 * D], F16)
            nc.sync.dma_start(out=bproj_sb, in_=bproj_d[:, :])
            wqkv_sb = const.tile([P, 4, 9 * D], F16)
            nc.sync.dma_start(out=wqkv_sb, in_=wqkv_d[:, :].rearrange("(c p) f -> p c f", p=P))
            bqkv_sb = const.tile([1, 9 * D], F16)
            nc.sync.dma_start(out=bqkv_sb, in_=bqkv_d[:, :])
            wh_sb = const.tile([P, 4, 768], F16)
            nc.sync.dma_start(out=wh_sb, in_=wh_d[:, :].rearrange("(c p) f -> p c f", p=P))
            bh_sb = const.tile([1, 256], F16)
            nc.sync.dma_start(out=bh_sb, in_=bh_d[:, :])
            wo_sb = const.tile([P, 4, D], F16)
            nc.sync.dma_start(out=wo_sb, in_=wo_d[:, :].rearrange("(c p) f -> p c f", p=P))
            bo_sb = const.tile([1, D], F16)
            nc.sync.dma_start(out=bo_sb, in_=bo_d[:, :])
            wg_sb = const.tile([P, 4, D], F16)
            nc.sync.dma_start(out=wg_sb, in_=wg_d[:, :].rearrange("(c p) f -> p c f", p=P))
            bg_sb = const.tile([1, D], F16)
            nc.sync.dma_start(out=bg_sb, in_=bg_d[:, :])
            wa2b_sb = const.tile([P, 3, 256], F32)
            nc.sync.dma_start(out=wa2b_sb, in_=bcast(wa2b_d[:, :]))
            ba2b_sb = const.tile([P, 1, 3], F32)
            nc.sync.dma_start(out=ba2b_sb, in_=bcast(ba2_d[:, :]))
            ba2b_sb = ba2b_sb[:, 0, :]
            aff_sb = None
            if aff_d is not None:
                aff_sb = const.tile([P, 4, D], F32)
                nc.sync.dma_start(out=aff_sb, in_=bcast(aff_d[:, :]))

            ident32 = const.tile([P, P], F32)
            make_identity(nc, ident32)
            ident16 = const.tile([P, P], F16)
            make_identity(nc, ident16)
            ones16 = const.tile([1, P], F16)
            nc.vector.memset(ones16, 1.0)
            eps_sb = const.tile([P, 1], F32)
            nc.vector.memset(eps_sb, EPS)

            isk = float(1.0 / np.sqrt(KD))

            def layernorm(zin, zout, aff_g, aff_b):
                """zout = LN(zin); aff_g/aff_b: optional [P, D] gamma/beta slices."""
                st6 = p_sm.tile([P, 6], F32, name="st6", tag="st6")
                nc.vector.bn_stats(out=st6[:], in_=zin[:])
                mv = p_sm.tile([P, 2], F32, name="mv", tag="mv")
                nc.vector.bn_aggr(out=mv[:], in_=st6[:])
                lnt = p_sm.tile([P, 1], F32, name="lnt", tag="lnt")
                nc.scalar.activation(out=lnt[:], in_=mv[:, 1:2], func=AF.Ln,
                                     bias=eps_sb[:], scale=1.0)
                rstd = p_sm.tile([P, 1], F32, name="rstd", tag="rstd")
                nc.scalar.activation(out=rstd[:], in_=lnt[:], func=AF.Exp, scale=-0.5)
                nc.vector.tensor_scalar(out=zout[:], in0=zin[:], scalar1=mv[:, 0:1],
                                        scalar2=rstd[:], op0=OP.subtract, op1=OP.mult)
                if aff_g is not None:
                    nc.vector.tensor_mul(out=zout[:], in0=zout[:], in1=aff_g)
                if aff_b is not None:
                    nc.vector.tensor_add(out=zout[:], in0=zout[:], in1=aff_b)

            def emitA(i):
                """Pre-attention phase: loads, transposes, proj/qkv/h matmuls,
                adaptive weights, scores, softmax, ctx."""
                r0 = i * P
                # 1. load raw inputs
                raw = [p_raw.tile([P, D], F32, name=f"raw{s}", tag=f"raw{s}") for s in range(3)]
                for s in range(3):
                    nc.sync.dma_start(out=raw[s][:], in_=x_ext[s][r0:r0 + P, :])

                # 2. rawT (fp16) via packed PE transposes
                rawT = []
                for s in range(3):
                    tp = ps_t32.tile([P, D], F32, name="tp32", tag="tp32")
                    for c in range(4):
                        nc.tensor.matmul(tp[:, ts(c)], lhsT=raw[s][:, ts(c)], rhs=ident32[:],
                                         is_transpose=True, start=(c == 0), stop=(c == 3))
                    rt = p_rt.tile([P, D], F16, name=f"rawT{s}", tag=f"rawT{s}")
                    nc.scalar.copy(out=rt[:], in_=tp[:])
                    rawT.append(rt)

                # 3. projections -> x_s (f32)
                x_t = []
                for s in range(3):
                    ps = ps_mm.tile([P, D], F32, name="mm", tag="mm")
                    for c in range(4):
                        nc.tensor.matmul(ps[:], lhsT=rawT[s][:, ts(c)],
                                         rhs=wproj_sb[:, c, ts(s, D)],
                                         start=(c == 0), stop=False)
                    nc.tensor.matmul(ps[:], lhsT=ones16[0:1, :], rhs=bproj_sb[0:1, ts(s, D)],
                                     start=False, stop=True)
                    xt = p_x.tile([P, D], F32, name=f"x{s}", tag=f"x{s}")
                    nc.scalar.copy(out=xt[:], in_=ps[:])
                    x_t.append(xt)

                # 4. qkv (fp16, bias folded)
                qkv_t = []
                for s in range(3):
                    qt = p_qkv.tile([P, 3 * D], F16, name=f"qkv{s}", tag=f"qkv{s}")
                    for g in range(3):
                        ps = ps_mm.tile([P, D], F32, name="mm", tag="mm")
                        col = s * 3 * D + g * D
                        for c in range(4):
                            nc.tensor.matmul(ps[:], lhsT=rawT[s][:, ts(c)],
                                             rhs=wqkv_sb[:, c, col:col + D],
                                             start=(c == 0), stop=False)
                        nc.tensor.matmul(ps[:], lhsT=ones16[0:1, :],
                                         rhs=bqkv_sb[0:1, col:col + D],
                                         start=False, stop=True)
                        nc.scalar.copy(out=qt[:, ts(g, D)], in_=ps[:])
                    qkv_t.append(qt)

                # 5. MLP hidden h = relu(sum_s raw_s @ A_s + bh)
                ps_h = ps_mm.tile([P, D], F32, name="mm", tag="mm")
                for s in range(3):
                    for c in range(4):
                        nc.tensor.matmul(ps_h[:, 0:256], lhsT=rawT[s][:, ts(c)],
                                         rhs=wh_sb[:, c, ts(s, 256)],
                                         start=(s == 0 and c == 0), stop=False)
                nc.tensor.matmul(ps_h[:, 0:256], lhsT=ones16[0:1, :], rhs=bh_sb[0:1, :],
                                 start=False, stop=True)
                h_t = p_sm.tile([P, 256], F32, name="h", tag="h")
                nc.scalar.activation(out=h_t[:], in_=ps_h[:, 0:256], func=AF.Relu)

                # 6. adaptive weights aw = softmax(h @ Wa2 + ba2)
                awl = p_sm.tile([P, 3], F32, name="awl", tag="awl")
                scr = p_sm.tile([P, 256], F32, name="scr", tag="scr")
                for j in range(3):
                    nc.vector.tensor_mul(out=scr[:], in0=h_t[:], in1=wa2b_sb[:, j, :])
                    nc.vector.tensor_reduce(out=awl[:, j:j + 1], in_=scr[:],
                                            axis=AX.X, op=OP.add)
                nc.vector.tensor_add(out=awl[:], in0=awl[:], in1=ba2b_sb[:])
                aw_e = p_sm.tile([P, 3], F32, name="awe", tag="awe")
                aw_sum = p_sm.tile([P, 1], F32, name="aws", tag="aws")
                nc.scalar.activation(out=aw_e[:], in_=awl[:], func=AF.Exp,
                                     accum_out=aw_sum[:])
                aw_r = p_sm.tile([P, 1], F32, name="awr", tag="awr")
                nc.vector.reciprocal(out=aw_r[:], in_=aw_sum[:])
                aw_t = p_sm.tile([P, 3], F32, name="aw", tag="aw")
                nc.vector.tensor_scalar_mul(out=aw_t[:], in0=aw_e[:], scalar1=aw_r[:])

                # 7. attention scores: per (q,s) pair, q*k product then per-head reduce.
                #    sc layout: [(q, s, h)] = col (q*3+s)*4 + h
                sc_t = p_sm.tile([P, 36], F32, name="sc", tag="sc")
                scr2 = p_sm.tile([P, D], F16, name="scr2", tag="scr2")
                for qi in range(3):
                    for si in range(3):
                        # q*k elementwise on GpSimd (frees DVE); per-head reduce on DVE
                        nc.gpsimd.tensor_mul(
                            out=scr2[:],
                            in0=qkv_t[qi][:, 0 * D:1 * D],
                            in1=qkv_t[si][:, 1 * D:2 * D])
                        j4 = (qi * 3 + si) * 4
                        nc.vector.tensor_reduce(
                            out=sc_t[:, j4:j4 + 4],
                            in_=scr2[:].rearrange("p (h k) -> p h k", h=H),
                            axis=AX.X, op=OP.add)

                # 8. softmax over s: e = exp(sc/sqrt(K)); attn = e / sum_s e
                e_t = p_sm.tile([P, 36], F32, name="e", tag="e")
                nc.scalar.activation(out=e_t[:], in_=sc_t[:], func=AF.Exp, scale=isk)
                e4 = e_t[:].rearrange("p (q s h) -> p q s h", q=3, s=3)
                ssum = p_sm.tile([P, 12], F32, name="ssum", tag="ssum")
                ss4 = ssum[:].rearrange("p (q h) -> p q h", q=3)
                nc.vector.tensor_add(out=ss4, in0=e4[:, :, 0, :], in1=e4[:, :, 1, :])
                nc.vector.tensor_add(out=ss4, in0=ss4, in1=e4[:, :, 2, :])
                rinv = p_sm.tile([P, 12], F32, name="rinv", tag="rinv")
                nc.vector.reciprocal(out=rinv[:], in_=ssum[:])
                attn = p_sm.tile([P, 36], F32, name="attn", tag="attn")
                a4 = attn[:].rearrange("p (q s h) -> p q s h", q=3, s=3)
                rb = rinv[:].rearrange("p (q h) -> p q h", q=3).unsqueeze(2).broadcast_to([P, 3, 3, H])
                nc.vector.tensor_mul(out=a4, in0=e4, in1=rb)

                # 9. ctx_q = sum_s bcast(attn[q,s,:]) * v_s  (big fused elementwise ops)
                ctx_t = [p_ctx.tile([P, D], F16, name=f"ctx{q}", tag=f"ctx{q}") for q in range(3)]
                ctmp = p_sm.tile([P, D], F16, name="ctmp", tag="ctmp")
                for qi in range(3):
                    for si in range(3):
                        ab = p_sm.tile([P, H, KD], F16, name=f"ab{si}", tag=f"ab{si}")
                        nc.scalar.copy(
                            out=ab[:],
                            in_=a4[:, qi, si, :].unsqueeze(-1).broadcast_to([P, H, KD]))
                        vsl = qkv_t[si][:, 2 * D:3 * D]
                        if si == 0:
                            nc.vector.tensor_mul(out=ctx_t[qi][:], in0=ab[:], in1=vsl)
                        else:
                            nc.vector.tensor_mul(out=ctmp[:], in0=ab[:], in1=vsl)
                            nc.vector.tensor_add(out=ctx_t[qi][:], in0=ctx_t[qi][:], in1=ctmp[:])

                return dict(r0=r0, x_t=x_t, ctx_t=ctx_t, aw_t=aw_t)

            def emitB(st):
                """Post-attention phase: ctx^T, o-proj, LN1, weighted, gate, LN2, store."""
                r0, x_t, ctx_t, aw_t = st["r0"], st["x_t"], st["ctx_t"], st["aw_t"]
                # 10. ctx^T via fp16 PE transposes
                ctxT = []
                for qi in range(3):
                    tp16 = ps_t16.tile([P, D], F16, name="tp16", tag="tp16")
                    for c in range(4):
                        nc.tensor.matmul(tp16[:, ts(c)], lhsT=ctx_t[qi][:, ts(c)], rhs=ident16[:],
                                         is_transpose=True, start=(c == 0), stop=(c == 3))
                    ct = p_ctx.tile([P, D], F16, name=f"ctxT{qi}", tag=f"ctxT{qi}")
                    nc.scalar.copy(out=ct[:], in_=tp16[:])
                    ctxT.append(ct)

                # 11. o-proj + residual + LN1 -> y_q
                y_t = []
                for qi in range(3):
                    ps = ps_mm.tile([P, D], F32, name="mm", tag="mm")
                    for c in range(4):
                        nc.tensor.matmul(ps[:], lhsT=ctxT[qi][:, ts(c)], rhs=wo_sb[:, c, :],
                                         start=(c == 0), stop=False)
                    nc.tensor.matmul(ps[:], lhsT=ones16[0:1, :], rhs=bo_sb[0:1, :],
                                     start=False, stop=True)
                    yraw = p_y.tile([P, D], F32, name=f"yraw{qi}", tag=f"yraw{qi}")
                    nc.vector.tensor_add(out=yraw[:], in0=ps[:], in1=x_t[qi][:])
                    yq = p_y.tile([P, D], F32, name=f"y{qi}", tag=f"y{qi}")
                    layernorm(yraw, yq,
                              aff_sb[:, 0, :] if need_aff1 else None,
                              aff_sb[:, 1, :] if need_aff1 else None)
                    y_t.append(yq)

                # 12. weighted = sum_q aw_q * y_q
                w_t = p_w.tile([P, D], F32, name="w", tag="w")
                nc.vector.tensor_scalar_mul(out=w_t[:], in0=y_t[0][:], scalar1=aw_t[:, 0:1])
                for qi in (1, 2):
                    nc.vector.scalar_tensor_tensor(out=w_t[:], in0=y_t[qi][:],
                                                   scalar=aw_t[:, qi:qi + 1], in1=w_t[:],
                                                   op0=OP.mult, op1=OP.add)

                # 13. weighted^T (f32 transpose, fp16 evict)
                tpw = ps_t32.tile([P, D], F32, name="tp32", tag="tp32")
                for c in range(4):
                    nc.tensor.matmul(tpw[:, ts(c)], lhsT=w_t[:, ts(c)], rhs=ident32[:],
                                     is_transpose=True, start=(c == 0), stop=(c == 3))
                wT = p_w.tile([P, D], F16, name="wT", tag="wT")
                nc.scalar.copy(out=wT[:], in_=tpw[:])

                # 14. gate = sigmoid(w @ Wg + bg) = exp(-ln(1+exp(-g)))
                ps_g = ps_mm.tile([P, D], F32, name="mm", tag="mm")
                for c in range(4):
                    nc.tensor.matmul(ps_g[:], lhsT=wT[:, ts(c)], rhs=wg_sb[:, c, :],
                                     start=(c == 0), stop=False)
                nc.tensor.matmul(ps_g[:], lhsT=ones16[0:1, :], rhs=bg_sb[0:1, :],
                                 start=False, stop=True)
                eg = p_w.tile([P, D], F32, name="eg", tag="eg")
                nc.scalar.activation(out=eg[:], in_=ps_g[:], func=AF.Exp, scale=-1.0)
                nc.vector.tensor_scalar_add(out=eg[:], in0=eg[:], scalar1=1.0)
                nc.scalar.activation(out=eg[:], in_=eg[:], func=AF.Ln)
                gate = p_w.tile([P, D], F32, name="gate", tag="gate")
                nc.scalar.activation(out=gate[:], in_=eg[:], func=AF.Exp, scale=-1.0)

                # 15. z = mp + gate*w ; out = LN2(z)
                z_t = p_w.tile([P, D], F32, name="z", tag="z")
                nc.vector.tensor_mul(out=z_t[:], in0=gate[:], in1=w_t[:])
                nc.vector.tensor_add(out=z_t[:], in0=z_t[:], in1=x_t[2][:])
                out_t = p_w.tile([P, D], F32, name="outt", tag="outt")
                layernorm(z_t, out_t,
                          aff_sb[:, 2, :] if need_aff2 else None,
                          aff_sb[:, 3, :] if need_aff2 else None)
                nc.sync.dma_start(out=out_ext[r0:r0 + P, :], in_=out_t[:])

            # Software pipeline with 1-tile lag: tile i+1's matmul-heavy phase A
            # is emitted before tile i's phase B, so the PE stream never stalls
            # waiting for tile i's DVE attention phase (keeps HAM warm).
            prev = None
            for i in range(ntiles):
                st = emitA(i)
                if prev is not None:
                    emitB(prev)
                prev = st
            emitB(prev)

    nc.finalize()
    return nc


def kernel(**inputs):
    global LAST_EXEC_TIME_NS, LAST_RESULTS
    inputs = {k: np.ascontiguousarray(np.asarray(v)) for k, v in inputs.items()}
    Bfull = inputs['frontier'].shape[0]
    assert Bfull % (NCORES * P) == 0
    R = Bfull // NCORES

    folded, ba2, aff = _fold(inputs)
    need_aff1 = not (np.allclose(aff[0], 1.0) and np.allclose(aff[1], 0.0))
    need_aff2 = not (np.allclose(aff[2], 1.0) and np.allclose(aff[3], 0.0))
    nc = _build(R, ba2, need_aff1, need_aff2)

    in_maps = []
    for c in range(NCORES):
        m = {n: inputs[n][c * R:(c + 1) * R] for n in ("frontier", "cross_robot", "map_feat")}
        m.update(folded)
        m["ba2"] = ba2[None, :]
        if need_aff1 or need_aff2:
            m["aff"] = aff
        in_maps.append(m)

    trace = bool(os.environ.get("KERNEL_TRACE"))
    res = run_bass_kernel_spmd(nc, in_maps, core_ids=list(range(NCORES)), trace=trace)
    LAST_EXEC_TIME_NS = res.exec_time_ns
    LAST_RESULTS = res
    out = np.concatenate([res.results[c]["out"] for c in range(NCORES)], axis=0)
    return out.astype(np.float32)


# revision 17
# speedup vs baseline: 1.2143x; 1.2143x over previous
"""Trainium2 Bass kernel: AdaptiveAttentionFusion, pure data-parallel on 8 NeuronCores.

Strategy:
  - Shard batch B=16384 across 8 cores (2048 rows each); weights replicated.
  - Host-side weight folding: input projections absorbed into QKV / MLP weights
    (q_f = frontier @ (Wf@Wq) + ...), so raw inputs feed all matmuls directly.
  - All matmul operands fp16 (validated 2.9e-4 rel err), fp32 PSUM accumulation.
  - Natural layout (rows on partitions); lhsT = transposed activations via PE
    transposes packed 4-per-PSUM-bank; biases folded in via K=1 ones-row matmuls.
  - Single ACT table set (natural_log_exp): softmax exp; LN rstd = exp(-0.5*ln(var+eps));
    sigmoid = exp(-ln(1+exp(-x))).
"""
import os
import numpy as np

import concourse.bacc as bacc
import concourse.bass as bass
import concourse.tile as tile
from concourse import mybir
from concourse.bass_utils import run_bass_kernel_spmd
from concourse.masks import make_identity

D, H, KD = 512, 4, 128
NCORES = 8
EPS = 1e-6
P = 128
F16 = mybir.dt.float16
F32 = mybir.dt.float32
AF = mybir.ActivationFunctionType
OP = mybir.AluOpType
AX = mybir.AxisListType

LAST_EXEC_TIME_NS = None
LAST_RESULTS = None


def ts(i, n=P):
    return slice(i * n, (i + 1) * n)


def _fold(inp):
    """Fold input projections into downstream weights. Returns (device arrays, ba2, affine)."""
    Ws = [inp['Wf'], inp['Wr'], inp['Wm']]
    bs = [inp['bf'], inp['br'], inp['bm']]
    Wqkv = np.concatenate([inp['Wq'].reshape(D, D), inp['Wk'].reshape(D, D),
                           inp['Wv'].reshape(D, D)], axis=1)          # [512, 1536]
    bqkv = np.concatenate([inp['bq'].reshape(-1), inp['bk'].reshape(-1),
                           inp['bv'].reshape(-1)])                    # [1536]
    f = {}
    f['wproj'] = np.concatenate(Ws, axis=1).astype(np.float16)        # [512, 1536]
    f['bproj'] = np.concatenate(bs)[None, :].astype(np.float16)       # [1, 1536]
    f['wqkv'] = np.concatenate([Ws[s] @ Wqkv for s in range(3)], axis=1).astype(np.float16)  # [512, 4608]
    f['bqkv'] = np.concatenate([bs[s] @ Wqkv + bqkv for s in range(3)])[None, :].astype(np.float16)
    A = [Ws[s] @ inp['Wa1'][s * D:(s + 1) * D] for s in range(3)]
    f['wh'] = np.concatenate(A, axis=1).astype(np.float16)            # [512, 768]
    f['bh'] = (inp['ba1'] + sum(bs[s] @ inp['Wa1'][s * D:(s + 1) * D]
                                for s in range(3)))[None, :].astype(np.float16)
    f['wo'] = inp['Wo'].reshape(D, D).astype(np.float16)
    f['bo'] = inp['bo'][None, :].astype(np.float16)
    f['wg'] = inp['Wg'].astype(np.float16)
    f['bg'] = inp['bg'][None, :].astype(np.float16)
    f['wa2b'] = np.ascontiguousarray(inp['Wa2'].T).astype(np.float32)  # [3, 256]
    aff = np.stack([inp['gamma1'], inp['beta1'], inp['gamma2'], inp['beta2']]).astype(np.float32)
    return f, inp['ba2'].astype(np.float32), aff


def _pin_act_table(nc):
    """Instance-level override of insert_act_table_loads: make
    natural_log_exp_and_others the only selectable ACT table set, so the
    kernel never thrashes table loads (it covers exp/ln/relu/copy/identity).
    Set ids stay aligned with act_info.json ordering."""
    import types
    import bass_rust as _bass_rust
    from concourse.hw_specs import get_activation_tables

    def patched(self):
        has_activation = any(
            isinstance(i, mybir.InstActivation)
            for b in self.main_func.blocks
            for i in b.instructions
        )
        if not has_activation:
            return
        tables = [
            (name, fns if name == "natural_log_exp_and_others" else set())
            for name, fns in get_activation_tables(self.m.arch).items()
        ]
        _bass_rust.insert_act_table_loads(self, tables)

    nc.insert_act_table_loads = types.MethodType(patched, nc)


def _build(R, ba2, need_aff1, need_aff2):
    ntiles = R // P
    nc = bacc.Bacc()
    _pin_act_table(nc)

    x_ext = [nc.declare_dram_parameter(n, [R, D], F32, isOutput=False)
             for n in ("frontier", "cross_robot", "map_feat")]
    wproj_d = nc.declare_dram_parameter("wproj", [D, 3 * D], F16, isOutput=False)
    bproj_d = nc.declare_dram_parameter("bproj", [1, 3 * D], F16, isOutput=False)
    wqkv_d = nc.declare_dram_parameter("wqkv", [D, 9 * D], F16, isOutput=False)
    bqkv_d = nc.declare_dram_parameter("bqkv", [1, 9 * D], F16, isOutput=False)
    wh_d = nc.declare_dram_parameter("wh", [D, 768], F16, isOutput=False)
    bh_d = nc.declare_dram_parameter("bh", [1, 256], F16, isOutput=False)
    wo_d = nc.declare_dram_parameter("wo", [D, D], F16, isOutput=False)
    bo_d = nc.declare_dram_parameter("bo", [1, D], F16, isOutput=False)
    wg_d = nc.declare_dram_parameter("wg", [D, D], F16, isOutput=False)
    bg_d = nc.declare_dram_parameter("bg", [1, D], F16, isOutput=False)
    wa2b_d = nc.declare_dram_parameter("wa2b", [3, 256], F32, isOutput=False)
    ba2_d = nc.declare_dram_parameter("ba2", [1, 3], F32, isOutput=False)
    aff_d = None
    if need_aff1 or need_aff2:
        aff_d = nc.declare_dram_parameter("aff", [4, D], F32, isOutput=False)
    out_ext = nc.declare_dram_parameter("out", [R, D], F32, isOutput=True)

    def bcast(ap, parts=P):
        """Partition-broadcast DMA source AP (stride-0 partition dim)."""
        return bass.AP(tensor=ap.tensor, offset=ap.offset, ap=[[0, parts]] + list(ap.ap))

    with tile.TileContext(nc) as tc:
        import contextlib
        with contextlib.ExitStack() as ctx:
            const = ctx.enter_context(tc.tile_pool(name="const", bufs=1))
            p_raw = ctx.enter_context(tc.tile_pool(name="p_raw", bufs=2))
            p_rt = ctx.enter_context(tc.tile_pool(name="p_rt", bufs=2))
            p_x = ctx.enter_context(tc.tile_pool(name="p_x", bufs=2))
            p_qkv = ctx.enter_context(tc.tile_pool(name="p_qkv", bufs=2))
            p_ctx = ctx.enter_context(tc.tile_pool(name="p_ctx", bufs=2))
            p_y = ctx.enter_context(tc.tile_pool(name="p_y", bufs=2))
            p_w = ctx.enter_context(tc.tile_pool(name="p_w", bufs=2))
            p_sm = ctx.enter_context(tc.tile_pool(name="p_sm", bufs=2))
            ps_t32 = ctx.enter_context(tc.tile_pool(name="ps_t32", bufs=2, space="PSUM"))
            ps_t16 = ctx.enter_context(tc.tile_pool(name="ps_t16", bufs=2, space="PSUM"))
            ps_mm = ctx.enter_context(tc.tile_pool(name="ps_mm", bufs=3, space="PSUM"))

            # ---- constants ----
            wproj_sb = const.tile([P, 4, 3 * D], F16)
            nc.sync.dma_start(out=wproj_sb, in_=wproj_d[:, :].rearrange("(c p) f -> p c f", p=P))
            bproj_sb = const.tile([1, 3 * D], F16)
            nc.sync.dma_start(out=bproj_sb, in_=bproj_d[:, :])
            wqkv_sb = const.tile([P, 4, 9 * D], F16)
            nc.sync.dma_start(out=wqkv_sb, in_=wqkv_d[:, :].rearrange("(c p) f -> p c f", p=P))
            bqkv_sb = const.tile([1, 9 * D], F16)
            nc.sync.dma_start(out=bqkv_sb, in_=bqkv_d[:, :])
            wh_sb = const.tile([P, 4, 768], F16)
            nc.sync.dma_start(out=wh_sb, in_=wh_d[:, :].rearrange("(c p) f -> p c f", p=P))
            bh_sb = const.tile([1, 256], F16)
            nc.sync.dma_start(out=bh_sb, in_=bh_d[:, :])
            wo_sb = const.tile([P, 4, D], F16)
            nc.sync.dma_start(out=wo_sb, in_=wo_d[:, :].rearrange("(c p) f -> p c f", p=P))
            bo_sb = const.tile([1, D], F16)
            nc.sync.dma_start(out=bo_sb, in_=bo_d[:, :])
            wg_sb = const.tile([P, 4, D], F16)
            nc.sync.dma_start(out=wg_sb, in_=wg_d[:, :].rearrange("(c p) f -> p c f", p=P))
            bg_sb = const.tile([1, D], F16)
            nc.sync.dma_start(out=bg_sb, in_=bg_d[:, :])
            wa2b_sb = const.tile([P, 3, 256], F32)
            nc.sync.dma_start(out=wa2b_sb, in_=bcast(wa2b_d[:, :]))
            ba2b_sb = const.tile([P, 1, 3], F32)
            nc.sync.dma_start(out=ba2b_sb, in_=bcast(ba2_d[:, :]))
            ba2b_sb = ba2b_sb[:, 0, :]
            aff_sb = None
            if aff_d is not None:
                aff_sb = const.tile([P, 4, D], F32)
                nc.sync.dma_start(out=aff_sb, in_=bcast(aff_d[:, :]))

            ident32 = const.tile([P, P], F32)
            make_identity(nc, ident32)
            ident16 = const.tile([P, P], F16)
            make_identity(nc, ident16)
            ones16 = const.tile([1, P], F16)
            nc.vector.memset(ones16, 1.0)
            eps_sb = const.tile([P, 1], F32)
            nc.vector.memset(eps_sb, EPS)

            isk = float(1.0 / np.sqrt(KD))

            def layernorm(zin, zout, aff_g, aff_b):
                """zout = LN(zin); aff_g/aff_b: optional [P, D] gamma/beta slices."""
                st6 = p_sm.tile([P, 6], F32, name="st6", tag="st6")
                nc.vector.bn_stats(out=st6[:], in_=zin[:])
                mv = p_sm.tile([P, 2], F32, name="mv", tag="mv")
                nc.vector.bn_aggr(out=mv[:], in_=st6[:])
                lnt = p_sm.tile([P, 1], F32, name="lnt", tag="lnt")
                nc.scalar.activation(out=lnt[:], in_=mv[:, 1:2], func=AF.Ln,
                                     bias=eps_sb[:], scale=1.0)
                rstd = p_sm.tile([P, 1], F32, name="rstd", tag="rstd")
                nc.scalar.activation(out=rstd[:], in_=lnt[:], func=AF.Exp, scale=-0.5)
                nc.vector.tensor_scalar(out=zout[:], in0=zin[:], scalar1=mv[:, 0:1],
                                        scalar2=rstd[:], op0=OP.subtract, op1=OP.mult)
                if aff_g is not None:
                    nc.vector.tensor_mul(out=zout[:], in0=zout[:], in1=aff_g)
                if aff_b is not None:
                    nc.vector.tensor_add(out=zout[:], in0=zout[:], in1=aff_b)

            def emitA(i):
                """Pre-attention phase: loads, transposes, proj/qkv/h matmuls,
                adaptive weights, scores, softmax, ctx."""
                r0 = i * P
                # 1. load raw inputs
                raw = [p_raw.tile([P, D], F32, name=f"raw{s}", tag=f"raw{s}") for s in range(3)]
                for s in range(3):
                    nc.sync.dma_start(out=raw[s][:], in_=x_ext[s][r0:r0 + P, :])

                # 2. rawT (fp16) via packed PE transposes
                rawT = []
                for s in range(3):
                    tp = ps_t32.tile([P, D], F32, name="tp32", tag="tp32")
                    for c in range(4):
                        nc.tensor.matmul(tp[:, ts(c)], lhsT=raw[s][:, ts(c)], rhs=ident32[:],
                                         is_transpose=True, start=(c == 0), stop=(c == 3))
                    rt = p_rt.tile([P, D], F16, name=f"rawT{s}", tag=f"rawT{s}")
                    nc.scalar.copy(out=rt[:], in_=tp[:])
                    rawT.append(rt)

                # 3. projections -> x_s (f32)
                x_t = []
                for s in range(3):
                    ps = ps_mm.tile([P, D], F32, name="mm", tag="mm")
                    for c in range(4):
                        nc.tensor.matmul(ps[:], lhsT=rawT[s][:, ts(c)],
                                         rhs=wproj_sb[:, c, ts(s, D)],
                                         start=(c == 0), stop=False)
                    nc.tensor.matmul(ps[:], lhsT=ones16[0:1, :], rhs=bproj_sb[0:1, ts(s, D)],
                                     start=False, stop=True)
                    xt = p_x.tile([P, D], F32, name=f"x{s}", tag=f"x{s}")
                    nc.scalar.copy(out=xt[:], in_=ps[:])
                    x_t.append(xt)

                # 4. qkv (fp16, bias folded)
                qkv_t = []
                for s in range(3):
                    qt = p_qkv.tile([P, 3 * D], F16, name=f"qkv{s}", tag=f"qkv{s}")
                    for g in range(3):
                        ps = ps_mm.tile([P, D], F32, name="mm", tag="mm")
                        col = s * 3 * D + g * D
                        for c in range(4):
                            nc.tensor.matmul(ps[:], lhsT=rawT[s][:, ts(c)],
                                             rhs=wqkv_sb[:, c, col:col + D],
                                             start=(c == 0), stop=False)
                        nc.tensor.matmul(ps[:], lhsT=ones16[0:1, :],
                                         rhs=bqkv_sb[0:1, col:col + D],
                                         start=False, stop=True)
                        nc.scalar.copy(out=qt[:, ts(g, D)], in_=ps[:])
                    qkv_t.append(qt)

                # 5. MLP hidden h = relu(sum_s raw_s @ A_s + bh)
                ps_h = ps_mm.tile([P, D], F32, name="mm", tag="mm")
                for s in range(3):
                    for c in range(4):
                        nc.tensor.matmul(ps_h[:, 0:256], lhsT=rawT[s][:, ts(c)],
                                         rhs=wh_sb[:, c, ts(s, 256)],
                                         start=(s == 0 and c == 0), stop=False)
                nc.tensor.matmul(ps_h[:, 0:256], lhsT=ones16[0:1, :], rhs=bh_sb[0:1, :],
                                 start=False, stop=True)
                h_t = p_sm.tile([P, 256], F32, name="h", tag="h")
                nc.scalar.activation(out=h_t[:], in_=ps_h[:, 0:256], func=AF.Relu)

                # 6. adaptive weights aw = softmax(h @ Wa2 + ba2)
                awl = p_sm.tile([P, 3], F32, name="awl", tag="awl")
                scr = p_sm.tile([P, 256], F32, name="scr", tag="scr")
                for j in range(3):
                    nc.vector.tensor_mul(out=scr[:], in0=h_t[:], in1=wa2b_sb[:, j, :])
                    nc.vector.tensor_reduce(out=awl[:, j:j + 1], in_=scr[:],
                                            axis=AX.X, op=OP.add)
                nc.vector.tensor_add(out=awl[:], in0=awl[:], in1=ba2b_sb[:])
                aw_e = p_sm.tile([P, 3], F32, name="awe", tag="awe")
                aw_sum = p_sm.tile([P, 1], F32, name="aws", tag="aws")
                nc.scalar.activation(out=aw_e[:], in_=awl[:], func=AF.Exp,
                                     accum_out=aw_sum[:])
                aw_r = p_sm.tile([P, 1], F32, name="awr", tag="awr")
                nc.vector.reciprocal(out=aw_r[:], in_=aw_sum[:])
                aw_t = p_sm.tile([P, 3], F32, name="aw", tag="aw")
                nc.vector.tensor_scalar_mul(out=aw_t[:], in0=aw_e[:], scalar1=aw_r[:])

                # 7. attention scores: per (q,s) pair, q*k product then per-head reduce.
                #    sc layout: [(q, s, h)] = col (q*3+s)*4 + h
                sc_t = p_sm.tile([P, 36], F32, name="sc", tag="sc")
                scr2 = p_sm.tile([P, D], F16, name="scr2", tag="scr2")
                for qi in range(3):
                    for si in range(3):
                        nc.vector.tensor_mul(
                            out=scr2[:],
                            in0=qkv_t[qi][:, 0 * D:1 * D],
                            in1=qkv_t[si][:, 1 * D:2 * D])
                        j4 = (qi * 3 + si) * 4
                        nc.vector.tensor_reduce(
                            out=sc_t[:, j4:j4 + 4],
                            in_=scr2[:].rearrange("p (h k) -> p h k", h=H),
                            axis=AX.X, op=OP.add)

                # 8. softmax over s: e = exp(sc/sqrt(K)); attn = e / sum_s e
                e_t = p_sm.tile([P, 36], F32, name="e", tag="e")
                nc.scalar.activation(out=e_t[:], in_=sc_t[:], func=AF.Exp, scale=isk)
                e4 = e_t[:].rearrange("p (q s h) -> p q s h", q=3, s=3)
                ssum = p_sm.tile([P, 12], F32, name="ssum", tag="ssum")
                ss4 = ssum[:].rearrange("p (q h) -> p q h", q=3)
                nc.vector.tensor_add(out=ss4, in0=e4[:, :, 0, :], in1=e4[:, :, 1, :])
                nc.vector.tensor_add(out=ss4, in0=ss4, in1=e4[:, :, 2, :])
                rinv = p_sm.tile([P, 12], F32, name="rinv", tag="rinv")
                nc.vector.reciprocal(out=rinv[:], in_=ssum[:])
                attn = p_sm.tile([P, 36], F32, name="attn", tag="attn")
                a4 = attn[:].rearrange("p (q s h) -> p q s h", q=3, s=3)
                rb = rinv[:].rearrange("p (q h) -> p q h", q=3).unsqueeze(2).broadcast_to([P, 3, 3, H])
                nc.vector.tensor_mul(out=a4, in0=e4, in1=rb)

                # 9. ctx accumulation per (q, head) with fp16 per-partition scalars
                ctx_t = [p_ctx.tile([P, D], F16, name=f"ctx{q}", tag=f"ctx{q}") for q in range(3)]
                for qi in range(3):
                    for h4 in range(H):
                        dst = ctx_t[qi][:, ts(h4, KD)]
                        vs = lambda si: qkv_t[si][:, 2 * D + h4 * KD: 2 * D + (h4 + 1) * KD]
                        col = lambda si: attn[:, (qi * 3 + si) * 4 + h4: (qi * 3 + si) * 4 + h4 + 1]
                        nc.vector.tensor_scalar_mul(out=dst, in0=vs(0), scalar1=col(0))
                        for si in (1, 2):
                            nc.vector.scalar_tensor_tensor(out=dst, in0=vs(si), scalar=col(si),
                                                           in1=dst, op0=OP.mult, op1=OP.add)

                return dict(r0=r0, x_t=x_t, ctx_t=ctx_t, aw_t=aw_t)

            def emitB(st):
                """Post-attention phase: ctx^T, o-proj, LN1, weighted, gate, LN2, store."""
                r0, x_t, ctx_t, aw_t = st["r0"], st["x_t"], st["ctx_t"], st["aw_t"]
                # 10. ctx^T via fp16 PE transposes
                ctxT = []
                for qi in range(3):
                    tp16 = ps_t16.tile([P, D], F16, name="tp16", tag="tp16")
                    for c in range(4):
                        nc.tensor.matmul(tp16[:, ts(c)], lhsT=ctx_t[qi][:, ts(c)], rhs=ident16[:],
                                         is_transpose=True, start=(c == 0), stop=(c == 3))
                    ct = p_ctx.tile([P, D], F16, name=f"ctxT{qi}", tag=f"ctxT{qi}")
                    nc.scalar.copy(out=ct[:], in_=tp16[:])
                    ctxT.append(ct)

                # 11. o-proj + residual + LN1 -> y_q
                y_t = []
                for qi in range(3):
                    ps = ps_mm.tile([P, D], F32, name="mm", tag="mm")
                    for c in range(4):
                        nc.tensor.matmul(ps[:], lhsT=ctxT[qi][:, ts(c)], rhs=wo_sb[:, c, :],
                                         start=(c == 0), stop=False)
                    nc.tensor.matmul(ps[:], lhsT=ones16[0:1, :], rhs=bo_sb[0:1, :],
                                     start=False, stop=True)
                    yraw = p_y.tile([P, D], F32, name=f"yraw{qi}", tag=f"yraw{qi}")
                    nc.vector.tensor_add(out=yraw[:], in0=ps[:], in1=x_t[qi][:])
                    yq = p_y.tile([P, D], F32, name=f"y{qi}", tag=f"y{qi}")
                    layernorm(yraw, yq,
                              aff_sb[:, 0, :] if need_aff1 else None,
                              aff_sb[:, 1, :] if need_aff1 else None)
                    y_t.append(yq)

                # 12. weighted = sum_q aw_q * y_q
                w_t = p_w.tile([P, D], F32, name="w", tag="w")
                nc.vector.tensor_scalar_mul(out=w_t[:], in0=y_t[0][:], scalar1=aw_t[:, 0:1])
                for qi in (1, 2):
                    nc.vector.scalar_tensor_tensor(out=w_t[:], in0=y_t[qi][:],
                                                   scalar=aw_t[:, qi:qi + 1], in1=w_t[:],
                                                   op0=OP.mult, op1=OP.add)

                # 13. weighted^T (f32 transpose, fp16 evict)
                tpw = ps_t32.tile([P, D], F32, name="tp32", tag="tp32")
                for c in range(4):
                    nc.tensor.matmul(tpw[:, ts(c)], lhsT=w_t[:, ts(c)], rhs=ident32[:],
                                     is_transpose=True, start=(c == 0), stop=(c == 3))
                wT = p_w.tile([P, D], F16, name="wT", tag="wT")
                nc.scalar.copy(out=wT[:], in_=tpw[:])

                # 14. gate = sigmoid(w @ Wg + bg) = exp(-ln(1+exp(-g)))
                ps_g = ps_mm.tile([P, D], F32, name="mm", tag="mm")
                for c in range(4):
                    nc.tensor.matmul(ps_g[:], lhsT=wT[:, ts(c)], rhs=wg_sb[:, c, :],
                                     start=(c == 0), stop=False)
                nc.tensor.matmul(ps_g[:], lhsT=ones16[0:1, :], rhs=bg_sb[0:1, :],
                                 start=False, stop=True)
                eg = p_w.tile([P, D], F32, name="eg", tag="eg")
                nc.scalar.activation(out=eg[:], in_=ps_g[:], func=AF.Exp, scale=-1.0)
                nc.vector.tensor_scalar_add(out=eg[:], in0=eg[:], scalar1=1.0)
                nc.scalar.activation(out=eg[:], in_=eg[:], func=AF.Ln)
                gate = p_w.tile([P, D], F32, name="gate", tag="gate")
                nc.scalar.activation(out=gate[:], in_=eg[:], func=AF.Exp, scale=-1.0)

                # 15. z = mp + gate*w ; out = LN2(z)
                z_t = p_w.tile([P, D], F32, name="z", tag="z")
                nc.vector.tensor_mul(out=z_t[:], in0=gate[:], in1=w_t[:])
                nc.vector.tensor_add(out=z_t[:], in0=z_t[:], in1=x_t[2][:])
                out_t = p_w.tile([P, D], F32, name="outt", tag="outt")
                layernorm(z_t, out_t,
                          aff_sb[:, 2, :] if need_aff2 else None,
                          aff_sb[:, 3, :] if need_aff2 else None)
                nc.sync.dma_start(out=out_ext[r0:r0 + P, :], in_=out_t[:])

            # Software pipeline with 1-tile lag: tile i+1's matmul-heavy phase A
            # is emitted before tile i's phase B, so the PE stream never stalls
            # waiting for tile i's DVE attention phase (keeps HAM warm).
            prev = None
            for i in range(ntiles):
                st = emitA(i)
                if prev is not None:
                    emitB(prev)
                prev = st
            emitB(prev)

    nc.finalize()
    return nc


def kernel(**inputs):
    global LAST_EXEC_TIME_NS, LAST_RESULTS
    inputs = {k: np.ascontiguousarray(np.asarray(v)) for k, v in inputs.items()}
    Bfull = inputs['frontier'].shape[0]
    assert Bfull % (NCORES * P) == 0
    R = Bfull // NCORES

    folded, ba2, aff = _fold(inputs)
    need_aff1 = not (np.allclose(aff[0], 1.0) and np.allclose(aff[1], 0.0))
    need_aff2 = not (np.allclose(aff[2], 1.0) and np.allclose(aff[3], 0.0))
    nc = _build(R, ba2, need_aff1, need_aff2)

    in_maps = []
    for c in range(NCORES):
        m = {n: inputs[n][c * R:(c + 1) * R] for n in ("frontier", "cross_robot", "map_feat")}
        m.update(folded)
        m["ba2"] = ba2[None, :]
        if need_aff1 or need_aff2:
            m["aff"] = aff
        in_maps.append(m)

    trace = bool(os.environ.get("KERNEL_TRACE"))
    res = run_bass_kernel_spmd(nc, in_maps, core_ids=list(range(NCORES)), trace=trace)
    LAST_EXEC_TIME_NS = res.exec_time_ns
    LAST_RESULTS = res
    out = np.concatenate([res.results[c]["out"] for c in range(NCORES)], axis=0)
    return out.astype(np.float32)
